# revision 1
# baseline (speedup 1.0000x reference)
"""Trainium2 Bass kernel for nn_APIHyperInputLayer (hypernetwork input layer).

Math (per branch, ally shown; enemy identical with F=28, E=11):
    h    = relu(feats @ w1 + b1)              [N, 64]
    w    = (h @ w2 + b2).reshape(N, F, 256)
    hid  = einsum('nf,nfo->no', feats, w)     [N, 256]
    out  = hid.reshape(B, E, 256).sum(1)      [B, 256]

Restructured to avoid materializing w (335MB):
    hid[n,o] = sum_{h,f} relu(h)[n,h] * feats[n,f] * W2[(h,f), o]
             + sum_f feats[n,f] * B2[f, o]
    with W2 = w2.reshape(64*F, 256), B2 = b2.reshape(F, 256), and
    G[(h,f), n] = relu(h)[n,h] * feats[n,f]   (khatri-rao product)
    => hid.T = W2.T @ G + B2.T @ feats.T  -- plain matmuls on the PE.

Per core (data-parallel over batch, 128 batches/core), blocked over n in
entity-aligned blocks of 32 batches (BW = 32*E columns):
  1. PE-transpose feats tiles -> featsT [F, n], replicated across PE
     row-group partitions for packed expansion matmuls.
  2. Expansion matmuls (2 or 4 concurrent via tile_position row groups):
     h_exp tile t = w1e_t.T @ featsT_rep -- partition p of tile t holds
     pre-relu h[4t + p//F] (b1 folded via a ones row when biases present).
  3. Fused relu*mult: G_t = max(h_exp_t, 0) * quad (DVE scalar_tensor_tensor
     from PSUM, or ACT relu + GPSIMD tensor_tensor for load balance).
  4. Big matmul: hidden[o, n] += W2_t.T @ G_t (16 k-tiles [+ bias matmul]).
  5. Entity reduction along free dim: hid[o, b] = sum_e hidden[o, b*E+e].
  6. Accumulating PE transposes (ally+enemy) -> out[b, o]; DMA out.

Matmuls run in float32r: fp32 bits at bf16 PE rate for N>=256 (HW rounds
operands; measured ~1.6e-4 rel err per K=128 matmul vs 5e-7 for true fp32,
which runs 4x slower).

The problem spec pins all four biases to zeros, so kernel() dispatches to a
no-bias program (4-way-packed expansion, no bias matmuls). If any bias is
nonzero it falls back to the bias-correct program (2-way packing, K=F+1
with a ones row, plus B2 accumulation matmuls).
"""

import sys

if "/opt/trn_rl_repo" not in sys.path:
    sys.path.insert(0, "/opt/trn_rl_repo")

import numpy as np

import concourse.mybir as mybir
from concourse import bacc
from concourse.tile import TileContext
from concourse.bass_utils import run_bass_kernel_spmd

F32 = mybir.dt.float32
F32R = mybir.dt.float32r
AX = mybir.AxisListType
ALU = mybir.AluOpType
ACTF = mybir.ActivationFunctionType

N_CORES = 8
B = 1024
OUT = 256
HID = 64

CFG = {
    "a": dict(F=32, E=10, TH=128),
    "e": dict(F=28, E=11, TH=112),
}
BC = B // N_CORES  # 128 batches per core
for _k, _c in CFG.items():
    _c["N"] = BC * _c["E"]           # rows per core (1280 / 1408)
    _c["HF"] = HID * _c["F"]         # contraction size (2048 / 1792)
    _c["KT"] = _c["HF"] // _c["TH"]  # 16 k-tiles, both branches
    _c["BW"] = 32 * _c["E"]          # n-block width (320 / 352)
    _c["NB"] = _c["N"] // _c["BW"]   # 4 blocks

# of each 16 k-tiles, this many run the fused relu*mult on DVE; the rest
# run ACT relu -> GPSIMD mult (engine load balancing).
DVE_OF_16 = 9


def _build_program(reps=1, bias=False, dve16=None, pack=None, bigrep=1):
    """bias=True: mathematically complete (b1 via ones row, b2 matmuls),
    2-way-packed expansion. bias=False: biases assumed zero, 4-way packing.
    pack: override row-group packing factor (1, 2 or 4). bigrep: timing
    probe, repeats big-matmul groups."""
    dve16 = DVE_OF_16 if dve16 is None else dve16
    if pack is None:
        pack = 2
    kext = (lambda c: c["F"] + 1) if bias else (lambda c: c["F"])
    nc = bacc.Bacc("TRN2", debug=False)

    dr = {}
    dr["af"] = nc.dram_tensor("af", [CFG["a"]["N"], 32], F32,
                              kind="ExternalInput")
    dr["ef"] = nc.dram_tensor("ef", [CFG["e"]["N"], 28], F32,
                              kind="ExternalInput")
    for br in ("a", "e"):
        c = CFG[br]
        dr[f"w1{br}"] = nc.dram_tensor(f"w1{br}", [kext(c), c["HF"]], F32R,
                                       kind="ExternalInput")
        dr[f"w2{br}"] = nc.dram_tensor(f"w2{br}", [c["HF"], 256], F32R,
                                       kind="ExternalInput")
        if bias:
            dr[f"b2{br}"] = nc.dram_tensor(f"b2{br}", [c["F"], 256], F32R,
                                           kind="ExternalInput")
    dr["ident"] = nc.dram_tensor("ident", [128, 128], F32,
                                 kind="ExternalInput")
    if bias:
        dr["ones"] = nc.dram_tensor("ones", [1, 1408], F32R,
                                    kind="ExternalInput")
    out_dram = nc.dram_tensor("out", [BC, 256], F32, kind="ExternalOutput")

    # row-group base partitions used by the packed expansion matmuls
    gstep = 128 // pack
    bases = [g * gstep for g in range(pack)]

    with TileContext(nc) as tc:
        with (
            tc.tile_pool(name="const", bufs=1) as cpool,
            tc.tile_pool(name="stg", bufs=4) as spool,
            tc.tile_pool(name="tmp", bufs=4) as tpool,
            tc.tile_pool(name="g", bufs=48) as gpool,
            tc.tile_pool(name="psum", bufs=1, space="PSUM") as ppool,
        ):
          for _rep in range(reps):
            ident = cpool.tile([128, 128], F32, name="ident")
            nc.sync.dma_start(ident, dr["ident"][:, :])

            w1sb, w2sb, b2sb, ext, quad = {}, {}, {}, {}, {}
            for br in ("a", "e"):
                c = CFG[br]
                w1sb[br] = cpool.tile([bases[-1] + kext(c), c["HF"]], F32R,
                                      name=f"w1{br}sb")
                w2sb[br] = [
                    cpool.tile([c["TH"], 256], F32R, name=f"w2{br}t{t}")
                    for t in range(c["KT"])
                ]
                if bias:
                    b2sb[br] = cpool.tile([c["F"], 256], F32R,
                                          name=f"b2{br}sb")
                ext[br] = cpool.tile([bases[-1] + kext(c), c["N"]], F32R,
                                     name=f"ext{br}")
                quad[br] = cpool.tile([4 * c["F"], c["N"]], F32R,
                                      name=f"quad{br}")

            # ---- feats transpose: [n,F] -> featsT [F,n] via PE transpose ----
            for br, feats_dram in (("a", dr["af"]), ("e", dr["ef"])):
                c = CFG[br]
                ntiles = (c["N"] + 127) // 128
                done = 0
                while done < ntiles:
                    batch = min(4, ntiles - done)
                    pft = ppool.tile([32, 512], F32, name="pft", tag="pex",
                                     bufs=5)
                    for j in range(batch):
                        i = done + j
                        stg = spool.tile([128, 32], F32, name="stg", tag="stg")
                        nc.sync.dma_start(
                            stg[:, : c["F"]],
                            feats_dram[i * 128:(i + 1) * 128, :],
                        )
                        nc.tensor.matmul(
                            pft[: c["F"], j * 128:(j + 1) * 128],
                            stg[:, : c["F"]],
                            ident,
                            is_transpose=True,
                            start=True,
                            stop=True,
                        )
                    nc.scalar.copy(
                        ext[br][: c["F"], done * 128: done * 128 + batch * 128],
                        pft[: c["F"], : batch * 128],
                    )
                    done += batch
                if bias:
                    # b1 fold: ones row at partition F
                    nc.sync.dma_start(
                        ext[br][c["F"]: c["F"] + 1, :], dr["ones"][:, : c["N"]]
                    )
                # row-group replicas for packed expansion matmuls
                for base in bases[1:]:
                    nc.sync.dma_start(
                        ext[br][base: base + kext(c), :],
                        ext[br][: kext(c), :],
                    )
                # quad = featsT replicated 4x, densely packed (mult operand)
                for g in range(4):
                    nc.scalar.dma_start(
                        quad[br][g * c["F"]:(g + 1) * c["F"], :],
                        ext[br][: c["F"], :],
                    )

            # first-layer weights (small): one copy per row group
            for br in ("a", "e"):
                c = CFG[br]
                for base in bases:
                    nc.sync.dma_start(
                        w1sb[br][base: base + kext(c), :], dr[f"w1{br}"][:, :]
                    )

            # bulk second-layer weights, alternating across both HWDGE rings
            for br in ("a", "e"):
                c = CFG[br]
                for t in range(c["KT"]):
                    eng = nc.sync if t % 2 == 0 else nc.scalar
                    eng.dma_start(
                        w2sb[br][t],
                        dr[f"w2{br}"][t * c["TH"]:(t + 1) * c["TH"], :],
                    )
                if bias:
                    nc.scalar.dma_start(b2sb[br], dr[f"b2{br}"][:, :])

            gtiles = {"a": {}, "e": {}}
            hid = {
                br: [cpool.tile([128, BC], F32, name=f"hid{br}{o}")
                     for o in range(2)]
                for br in ("a", "e")
            }

            def produce_g(br, bb):
                c = CFG[br]
                lo, w = bb * c["BW"], c["BW"]
                kf = kext(c)
                for t in range(c["KT"]):
                    gt = gpool.tile([c["TH"], c["BW"]], F32R,
                                    name=f"g{br}{t}_{bb}", tag="g")
                    gtiles[br][(t, bb)] = gt
                    pex = ppool.tile([c["TH"], c["BW"]], F32, name="pex",
                                     tag="pex", bufs=5)
                    base = bases[t % pack]
                    nc.tensor.matmul(
                        pex,
                        w1sb[br][base: base + kf,
                                 t * c["TH"]:(t + 1) * c["TH"]],
                        ext[br][base: base + kf, lo: lo + w],
                        start=True,
                        stop=True,
                        tile_position=(base, 0) if pack > 1 else None,
                    )
                    if t % 16 < dve16:
                        nc.vector.scalar_tensor_tensor(
                            gt, pex, 0.0, quad[br][:, lo: lo + w],
                            op0=ALU.max, op1=ALU.mult,
                        )
                    else:
                        tmp = tpool.tile([c["TH"], c["BW"]], F32, name="tmp",
                                         tag="tmp")
                        nc.scalar.activation(tmp, pex, ACTF.Relu)
                        nc.gpsimd.tensor_tensor(
                            gt, tmp, quad[br][:, lo: lo + w], op=ALU.mult,
                        )

            def big_matmul(br, bb):
                c = CFG[br]
                lo, w = bb * c["BW"], c["BW"]
                for o_rep in range(2 * bigrep):
                    o = o_rep % 2
                    pbig = ppool.tile([128, CFG["e"]["BW"]], F32, name="pbig",
                                      tag="pbig", bufs=3)
                    for t in range(c["KT"]):
                        nc.tensor.matmul(
                            pbig[:, :w],
                            w2sb[br][t][:, o * 128:(o + 1) * 128],
                            gtiles[br][(t, bb)],
                            start=(t == 0),
                            stop=(t == c["KT"] - 1 and not bias),
                        )
                    if bias:
                        nc.tensor.matmul(
                            pbig[:, :w],
                            b2sb[br][:, o * 128:(o + 1) * 128],
                            ext[br][: c["F"], lo: lo + w],
                            start=False,
                            stop=True,
                        )
                    nc.vector.tensor_reduce(
                        hid[br][o][:, bb * 32:(bb + 1) * 32],
                        pbig[:, :w].rearrange("p (b e) -> p b e", e=c["E"]),
                        axis=AX.X,
                        op=ALU.add,
                    )

            # interleaved emission: keep PE fed while elementwise catches up
            steps = [
                ("a", "p", 0), ("a", "p", 1), ("a", "b", 0), ("a", "p", 2),
                ("a", "b", 1), ("a", "p", 3), ("e", "p", 0), ("a", "b", 2),
                ("e", "p", 1), ("a", "b", 3), ("e", "b", 0), ("e", "p", 2),
                ("e", "b", 1), ("e", "p", 3), ("e", "b", 2), ("e", "b", 3),
            ]
            for br, kind, bb in steps:
                if kind == "p":
                    produce_g(br, bb)
                else:
                    big_matmul(br, bb)

            # ---- final: out[b, o] = sum_br hid[br].T via accumulating
            # PE transposes ----
            pout = ppool.tile([128, 256], F32, name="pout", tag="pbig", bufs=3)
            for o in range(2):
                for i, br in enumerate(("a", "e")):
                    nc.tensor.matmul(
                        pout[:, o * 128:(o + 1) * 128],
                        hid[br][o],
                        ident,
                        is_transpose=True,
                        start=(i == 0),
                        stop=(i == 1),
                    )
            out_sb = cpool.tile([BC, 256], F32, name="out_sb")
            nc.scalar.copy(out_sb, pout)
            nc.sync.dma_start(out_dram[:, :], out_sb)

    nc.compile()
    return nc


def _host_inputs(ally_features, enemy_features, wa1, ba1, wa2, ba2,
                 we1, be1, we2, be2, bias=False):
    """Build the per-core in_maps (weights replicated, feats sharded)."""
    f32 = np.float32
    shared = {
        "w2a": np.ascontiguousarray(np.asarray(wa2).reshape(2048, 256),
                                    dtype=f32),
        "w2e": np.ascontiguousarray(np.asarray(we2).reshape(1792, 256),
                                    dtype=f32),
        "ident": np.eye(128, dtype=f32),
    }
    for key, w1, b1, F in (("w1a", wa1, ba1, 32), ("w1e", we1, be1, 28)):
        w1r = np.repeat(np.asarray(w1, dtype=f32), F, axis=1)
        if bias:
            w1x = np.empty((F + 1, HID * F), dtype=f32)
            w1x[:F] = w1r
            w1x[F] = np.repeat(np.asarray(b1, dtype=f32), F)
            shared[key] = w1x
        else:
            shared[key] = np.ascontiguousarray(w1r)
    if bias:
        shared["b2a"] = np.ascontiguousarray(
            np.asarray(ba2).reshape(32, 256), dtype=f32)
        shared["b2e"] = np.ascontiguousarray(
            np.asarray(be2).reshape(28, 256), dtype=f32)
        shared["ones"] = np.ones((1, 1408), dtype=f32)

    af = np.ascontiguousarray(ally_features, dtype=f32)
    ef = np.ascontiguousarray(enemy_features, dtype=f32)
    na, ne = CFG["a"]["N"], CFG["e"]["N"]
    in_maps = []
    for cix in range(N_CORES):
        m = dict(shared)
        m["af"] = np.ascontiguousarray(af[cix * na:(cix + 1) * na])
        m["ef"] = np.ascontiguousarray(ef[cix * ne:(cix + 1) * ne])
        in_maps.append(m)
    return in_maps


_nc_cache = {}


def _get_nc(reps=1, **kw):
    key = (reps, tuple(sorted(kw.items())))
    if key not in _nc_cache:
        _nc_cache[key] = _build_program(reps, **kw)
    return _nc_cache[key]


def kernel(**inputs) -> np.ndarray:
    bias = any(
        np.any(np.asarray(inputs[k])) for k in ("ba1", "ba2", "be1", "be2")
    )
    nc = _get_nc(bias=bias)
    in_maps = _host_inputs(bias=bias, **inputs)
    res = run_bass_kernel_spmd(nc, in_maps, core_ids=list(range(N_CORES)))
    return np.concatenate([r["out"] for r in res.results], axis=0)


if __name__ == "__main__":
    import reference

    inputs = {k: np.asarray(v) for k, v in reference.setup_inputs().items()}
    expected = np.asarray(reference.reference(**inputs))
    actual = kernel(**inputs)
    denom = np.abs(expected).max()
    print("abs max err:", np.abs(actual - expected).max())
    print("rel err:", np.abs(actual - expected).max() / denom)



# revision 21
# speedup vs baseline: 6.8941x; 6.8941x over previous
"""Trainium2 Bass kernel for nn_APIHyperInputLayer (hypernetwork input layer).

Math (per branch, ally shown; enemy identical with F=28, E=11):
    h    = relu(feats @ w1 + b1)              [N, 64]
    w    = (h @ w2 + b2).reshape(N, F, 256)
    hid  = einsum('nf,nfo->no', feats, w)     [N, 256]
    out  = hid.reshape(B, E, 256).sum(1)      [B, 256]

Key restructurings:
  1. Avoid materializing w (335MB):
       hid.T = W2.T @ G,  G[(j,f), n] = relu(h)[n,j] * feats[n,f]
     with W2 = w2.reshape(64*F, 256) (j-major, f-fast).
  2. The entity sum commutes past W2:
       out[b, :] = sum_k W2[k, :] * Gs[k, b],  Gs[k, b] = sum_e G[k, (b,e)]
     so the big matmul contracts against the entity-POOLED Gs [HF, BC]
     instead of G [HF, N] — ~10x less PE work — and with Gs as the
     stationary operand the result lands directly as out[b, o] in PSUM,
     accumulating both branches into one [128, 256] tile (no final
     transposes / adds).

Per core (data-parallel over batch, BC=128 batches/core):
  - feats arrive host-transposed+replicated-ready as bf16 [F, N]; SBUF
    replicas at row-group bases 0/32/64/96 serve the 4-way tile_position
    packed expansion matmuls (for ally F=32 the same tile doubles as the
    elementwise multiplicand; enemy needs a separate F-strided quad).
  - w1 arrives host-packed by row-group base: base b holds the k-tiles
    t = b (mod 4), so one dense [128, 4*TH] DMA replaces 4 replicas.
  - expansion (bf16): pex(t) [TH, BW] = w1-slice.T @ featsT-slice per
    n-block (BW = 32 batches * E cols), 4 concurrent via row groups.
  - G(t) bf16 = max(pex, 0) * featsT-replica — routed across DVE
    (fused scalar_tensor_tensor), Pool (fused stt), and ACT relu + Pool
    mult, per ROUTE, to balance engine load.
  - Gs_f32[t][:, bb] = grouped entity reduce of G (DVE, 3D AP) then one
    ACT convert to bf16 per k-tile.
  - big matmul: out_psum[128b, 256o] += Gs_bf16[t].T @ W2[t], 32
    accumulating matmuls over both branches' k-tiles; copy + DMA out.

bf16 everywhere off the PE accumulators keeps total rel err ~1e-3
(tolerance 2e-2) while halving DMA and enabling 2x elementwise rates.

Biases are pinned to zero in this problem spec; the bias=True fallback
(2-way packing, K=F+1 ones-row for b1, entity-pooled feats @ B2 term
for b2) keeps the kernel mathematically complete for nonzero biases.
"""

import sys

if "/opt/trn_rl_repo" not in sys.path:
    sys.path.insert(0, "/opt/trn_rl_repo")

import numpy as np
import ml_dtypes

import concourse.mybir as mybir
from concourse import bacc
from concourse.tile import TileContext
from concourse.bass_utils import run_bass_kernel_spmd

F32 = mybir.dt.float32
BF16 = mybir.dt.bfloat16
AX = mybir.AxisListType
ALU = mybir.AluOpType
ACTF = mybir.ActivationFunctionType
BF = ml_dtypes.bfloat16

N_CORES = 8
B = 1024
OUT = 256
HID = 64

CFG = {
    "a": dict(F=32, E=10, TH=128),
    "e": dict(F=28, E=11, TH=112),
}
BC = B // N_CORES  # 128 batches per core
for _k, _c in CFG.items():
    _c["N"] = BC * _c["E"]           # rows per core (1280 / 1408)
    _c["HF"] = HID * _c["F"]         # contraction size (2048 / 1792)
    _c["KT"] = _c["HF"] // _c["TH"]  # 16 k-tiles, both branches
    _c["EH"] = _c["E"] // 2          # halved entity count (5 / 5)
    # entity-aligned n-blocks sized near the 512-f32 PSUM bank limit
    _bw = 32 * _c["E"]
    if _c["E"] == 10:
        _c["BLOCKS"] = [(0, 480), (480, 480), (960, 320)]
    else:
        _c["BLOCKS"] = [(0, 484), (484, 484), (968, 440)]
    assert sum(w for _, w in _c["BLOCKS"]) == _c["N"]

# Elementwise routing. Pool/GPSIMD cannot read PSUM (BIR verifier rule),
# so PSUM evacuation of the expansion output is split between DVE (fused
# relu*mult stt) and ACT (relu to bf16) + Pool (mult, SBUF-only): a
# block goes to DVE when (block_index % dve_den) < dve_num. Every k-tile
# is then entity-HALVED on Pool (strided pair-add, bf16). The first
# poff_a/poff_e k-tiles per branch are pooled by the PE (EH+1 strided-
# stationary accumulating matmuls into the output PSUM); the rest get a
# DVE grouped reduce (+ leftover add for odd E) and an ACT convert.
ROUTE = dict(dve_num=1, dve_den=2, poff_a=13, poff_e=12)


def _build_program(reps=1, bias=False, route=None):
    route = dict(ROUTE if route is None else route)
    pack = 2 if bias else 4
    gstep = 128 // pack
    bases = list(range(0, 128, gstep))
    kext = (lambda c: c["F"] + 1) if bias else (lambda c: c["F"])

    nc = bacc.Bacc("TRN2", debug=False)

    dr = {}
    for br in ("a", "e"):
        c = CFG[br]
        dr[f"x{br}"] = nc.dram_tensor(f"x{br}", [c["F"], c["N"]], BF16,
                                      kind="ExternalInput")
        dr[f"w1{br}"] = nc.dram_tensor(
            f"w1{br}", [bases[-1] + kext(c), (c["KT"] // pack) * c["TH"]],
            BF16, kind="ExternalInput")
        dr[f"w2{br}"] = nc.dram_tensor(f"w2{br}", [c["HF"], 256], BF16,
                                       kind="ExternalInput")
        if bias:
            dr[f"b2{br}"] = nc.dram_tensor(f"b2{br}", [c["F"], 256], BF16,
                                           kind="ExternalInput")
    if bias:
        dr["ones"] = nc.dram_tensor("ones", [1, 1408], BF16,
                                    kind="ExternalInput")
    out_dram = nc.dram_tensor("out", [BC, 256], F32, kind="ExternalOutput")

    with TileContext(nc) as tc:
        with (
            tc.tile_pool(name="const", bufs=1) as cpool,
            tc.tile_pool(name="tmp", bufs=4) as tpool,
            tc.tile_pool(name="g", bufs=12) as gpool,
            tc.tile_pool(name="psum", bufs=1, space="PSUM") as ppool,
        ):
          for _rep in range(reps):
            w1sb, w2sb, ext, quad, b2sb = {}, {}, {}, {}, {}
            for br in ("a", "e"):
                c = CFG[br]
                w1sb[br] = cpool.tile(
                    [bases[-1] + kext(c), (c["KT"] // pack) * c["TH"]],
                    BF16, name=f"w1{br}sb")
                w2sb[br] = [
                    cpool.tile([c["TH"], 256], BF16, name=f"w2{br}t{t}")
                    for t in range(c["KT"])
                ]
                ext[br] = cpool.tile([bases[-1] + kext(c), c["N"]], BF16,
                                     name=f"ext{br}")
                if bias or c["F"] != gstep:
                    quad[br] = cpool.tile([4 * c["F"], c["N"]], BF16,
                                          name=f"quad{br}")
                if bias:
                    b2sb[br] = cpool.tile([c["F"], 256], BF16,
                                          name=f"b2{br}sb")

            # ---- loads: w1 + featsT replicas first, split across both
            # HWDGE rings so the first expansion matmuls unblock early;
            # bulk w2 after, alternating rings ----
            # ACT is compute-loaded (relu path), so its HWDGE ring carries
            # only a small early slice (w1 + first replica per branch);
            # everything else issues from the SP ring.
            for br in ("a", "e"):
                c = CFG[br]
                nc.scalar.dma_start(w1sb[br], dr[f"w1{br}"][:, :])
                for i, base in enumerate(bases):
                    eng = nc.scalar if i == 0 else nc.sync
                    eng.dma_start(
                        ext[br][base: base + c["F"], :], dr[f"x{br}"][:, :])
                if bias:
                    for base in bases:
                        nc.scalar.dma_start(
                            ext[br][base + c["F"]: base + c["F"] + 1, :],
                            dr["ones"][:, : c["N"]])
                if br in quad:
                    for g in range(4):
                        nc.sync.dma_start(
                            quad[br][g * c["F"]:(g + 1) * c["F"], :],
                            dr[f"x{br}"][:, :])
                if bias:
                    nc.scalar.dma_start(b2sb[br], dr[f"b2{br}"][:, :])
            for br in ("a", "e"):
                c = CFG[br]
                for t in range(c["KT"]):
                    eng = nc.scalar if t % 4 == 3 else nc.sync
                    eng.dma_start(
                        w2sb[br][t],
                        dr[f"w2{br}"][t * c["TH"]:(t + 1) * c["TH"], :])

            poff = {"a": route["poff_a"], "e": route["poff_e"]}
            gfull, ghalf, gs32, gs16 = {}, {}, {}, {}
            for br in ("a", "e"):
                c = CFG[br]
                gfull[br] = [gpool.tile([c["TH"], c["N"]], BF16,
                                        name=f"g{br}{t}", tag="g")
                             for t in range(c["KT"])]
                ghalf[br] = [gpool.tile([c["TH"], BC * c["EH"]], BF16,
                                        name=f"gh{br}{t}", tag="gh")
                             for t in range(c["KT"])]
                gs32[br] = [cpool.tile([c["TH"], BC], F32,
                                       name=f"gs32{br}{t}")
                            if t >= poff[br] else None
                            for t in range(c["KT"])]
                gs16[br] = [cpool.tile([c["TH"], BC], BF16,
                                       name=f"gs16{br}{t}")
                            if t >= poff[br] else None
                            for t in range(c["KT"])]

            def mult_operand(br, lo, w):
                c = CFG[br]
                src = quad[br] if br in quad else ext[br]
                return src[: c["TH"], lo: lo + w]

            # ---- main pipeline: k-tile outer, n-block inner. All 4
            # blocks of a k-tile share one loaded weight slice, the wide
            # entity-reduce fires per k-tile (spreading DVE load evenly),
            # and output-PSUM matmuls are woven in LAG k-tiles behind so
            # the in-order PE queue never stalls the expansion feed. ----
            pout = ppool.tile([BC, 256], F32, name="pout", tag="pout")
            n_pout = sum(
                poff[br] * (CFG[br]["EH"] + CFG[br]["E"] % 2)
                + (CFG[br]["KT"] - poff[br])
                for br in ("a", "e"))
            pout_emitted = [0]
            pending = []

            def emit_pout(br, t):
                c = CFG[br]
                if t < poff[br]:
                    lhss = [ghalf[br][t].rearrange(
                        "p (b e) -> p b e", e=c["EH"])[:, :, e]
                        for e in range(c["EH"])]
                    if c["E"] % 2:  # odd E: unpaired entity from G
                        lhss.append(gfull[br][t].rearrange(
                            "p (b e) -> p b e", e=c["E"])[:, :, c["E"] - 1])
                else:
                    lhss = [gs16[br][t]]
                for lhs in lhss:
                    pout_emitted[0] += 1
                    nc.tensor.matmul(
                        pout, lhs, w2sb[br][t],
                        start=(pout_emitted[0] == 1),
                        stop=(pout_emitted[0] == n_pout and not bias),
                    )

            gi = [0]
            LAG = 2
            for br in ("a", "e"):
                c = CFG[br]
                kf = kext(c)
                for t in range(c["KT"]):
                    base = bases[t % pack]
                    for lo, w in c["BLOCKS"]:
                        pex = ppool.tile([c["TH"], w], F32,
                                         name="pex", tag="pex", bufs=6)
                        nc.tensor.matmul(
                            pex,
                            w1sb[br][base: base + kf,
                                     (t // pack) * c["TH"]:
                                     (t // pack + 1) * c["TH"]],
                            ext[br][base: base + kf, lo: lo + w],
                            start=True,
                            stop=True,
                            tile_position=(base, 0),
                        )
                        gt = gfull[br][t][:, lo: lo + w]
                        if gi[0] % route["dve_den"] < route["dve_num"]:
                            nc.vector.scalar_tensor_tensor(
                                gt, pex, 0.0, mult_operand(br, lo, w),
                                op0=ALU.max, op1=ALU.mult)
                        else:
                            tmp = tpool.tile([c["TH"], w], BF16,
                                             name="tmp", tag="tmp")
                            nc.scalar.activation(tmp, pex, ACTF.Relu)
                            nc.gpsimd.tensor_tensor(
                                gt, tmp, mult_operand(br, lo, w),
                                op=ALU.mult)
                        gi[0] += 1
                    # entity pair-halving on Pool (strided bf16 adds)
                    g3 = gfull[br][t].rearrange("p (b e) -> p b e",
                                                e=c["E"])
                    eh = c["EH"]
                    nc.gpsimd.tensor_tensor(
                        ghalf[br][t].rearrange("p (b e) -> p b e", e=eh),
                        g3[:, :, 0:eh], g3[:, :, eh:2 * eh], op=ALU.add)
                    if t >= poff[br]:
                        # grouped entity reduce on halves (DVE-only op)
                        nc.vector.tensor_reduce(
                            gs32[br][t],
                            ghalf[br][t].rearrange("p (b e) -> p b e",
                                                   e=eh),
                            axis=AX.X, op=ALU.add)
                        if c["E"] % 2:
                            nc.vector.tensor_tensor(
                                gs32[br][t], gs32[br][t],
                                g3[:, :, c["E"] - 1], op=ALU.add)
                        nc.scalar.copy(gs16[br][t], gs32[br][t])
                    pending.append((br, t))
                    if len(pending) > LAG:
                        emit_pout(*pending.pop(0))
            while pending:
                emit_pout(*pending.pop(0))
            if bias:
                # out += sum_e feats[., f] @ B2[f, :]  via entity-pooled feats
                for j, br in enumerate(("a", "e")):
                    c = CFG[br]
                    fsum = cpool.tile([c["F"], BC], F32, name=f"fsum{br}")
                    nc.vector.tensor_reduce(
                        fsum,
                        ext[br][: c["F"], :].rearrange(
                            "p (b e) -> p b e", e=c["E"]),
                        axis=AX.X, op=ALU.add)
                    fsum16 = cpool.tile([c["F"], BC], BF16,
                                        name=f"fsum16{br}")
                    nc.scalar.copy(fsum16, fsum)
                    nc.tensor.matmul(
                        pout, fsum16, b2sb[br],
                        start=False, stop=(j == 1))

            out_sb = cpool.tile([BC, 256], F32, name="out_sb")
            nc.scalar.copy(out_sb, pout)
            nc.sync.dma_start(out_dram[:, :], out_sb)

    nc.compile()
    return nc


def _pack_w1(w1, b1, F, TH, pack, bias):
    """Host-pack first-layer weights by row-group base: base b holds the
    k-tiles t congruent to b (mod pack), densely."""
    w1r = np.repeat(np.asarray(w1, dtype=np.float32), F, axis=1)  # [F, HF]
    kf = F + 1 if bias else F
    rows = (pack - 1) * (128 // pack) + kf
    kt = w1r.shape[1] // TH
    packed = np.zeros((rows, (kt // pack) * TH), dtype=np.float32)
    for t in range(kt):
        base = (t % pack) * (128 // pack)
        ti = t // pack
        packed[base: base + F, ti * TH:(ti + 1) * TH] = \
            w1r[:, t * TH:(t + 1) * TH]
        if bias:
            packed[base + F, ti * TH:(ti + 1) * TH] = np.repeat(
                np.asarray(b1, dtype=np.float32), F)[t * TH:(t + 1) * TH]
    return packed.astype(BF)


def _host_inputs(ally_features, enemy_features, wa1, ba1, wa2, ba2,
                 we1, be1, we2, be2, bias=False):
    pack = 2 if bias else 4
    shared = {
        "w1a": _pack_w1(wa1, ba1, 32, CFG["a"]["TH"], pack, bias),
        "w1e": _pack_w1(we1, be1, 28, CFG["e"]["TH"], pack, bias),
        "w2a": np.asarray(wa2, dtype=np.float32).reshape(2048, 256)
                 .astype(BF),
        "w2e": np.asarray(we2, dtype=np.float32).reshape(1792, 256)
                 .astype(BF),
    }
    if bias:
        shared["b2a"] = np.asarray(ba2, np.float32).reshape(32, 256)\
            .astype(BF)
        shared["b2e"] = np.asarray(be2, np.float32).reshape(28, 256)\
            .astype(BF)
        shared["ones"] = np.ones((1, 1408), dtype=BF)

    af = np.asarray(ally_features, dtype=np.float32)
    ef = np.asarray(enemy_features, dtype=np.float32)
    na, ne = CFG["a"]["N"], CFG["e"]["N"]
    in_maps = []
    for cix in range(N_CORES):
        m = dict(shared)
        m["xa"] = np.ascontiguousarray(
            af[cix * na:(cix + 1) * na].T).astype(BF)
        m["xe"] = np.ascontiguousarray(
            ef[cix * ne:(cix + 1) * ne].T).astype(BF)
        in_maps.append(m)
    return in_maps


_nc_cache = {}


def _get_nc(reps=1, **kw):
    key = (reps, tuple(sorted(kw.items())))
    if key not in _nc_cache:
        _nc_cache[key] = _build_program(reps, **kw)
    return _nc_cache[key]


def kernel(**inputs) -> np.ndarray:
    bias = any(
        np.any(np.asarray(inputs[k])) for k in ("ba1", "ba2", "be1", "be2")
    )
    nc = _get_nc(bias=bias)
    in_maps = _host_inputs(bias=bias, **inputs)
    res = run_bass_kernel_spmd(nc, in_maps, core_ids=list(range(N_CORES)))
    return np.concatenate([r["out"] for r in res.results], axis=0)


if __name__ == "__main__":
    import reference

    inputs = {k: np.asarray(v) for k, v in reference.setup_inputs().items()}
    expected = np.asarray(reference.reference(**inputs))
    actual = kernel(**inputs)
    denom = np.abs(expected).max()
    print("abs max err:", np.abs(actual - expected).max())
    print("rel err:", np.abs(actual - expected).max() / denom)
